# revision 15
# baseline (speedup 1.0000x reference)
"""Trainium2 Bass kernel for nn_Listener (LSTM listener + dense encoders).

Reference computation (per full batch B=512):
    emb = embed_table[message]                       # [B, T, 512]
    LSTM over T=128 steps, HIDDEN=1024:
        gated = [x_t, h] @ W_cell + b_cell           # [B, 4096] (i, g, f, o)
        f = sigmoid(f + 1); c = f*c + sigmoid(i)*tanh(g); h = sigmoid(o)*tanh(c)
    images_encoded = images @ W_img + b_img          # [B, 1024]
    hidden_encoded = h @ W_hid + b_hid               # [B, 1024]
    returns (images_encoded, hidden_encoded)

Strategy (8 NeuronCores, data-parallel over batch, 64 rows/core):
  * Embedding lookup + x-projection fold into one table:
        M2 = embed_table @ W_cell[:512] + b_cell  (+1 on the f columns)
    stored bf16; per step the rows are gathered by token id and injected
    into PSUM with a k=128 bf16 selection matmul that also seeds the
    accumulation groups.
  * Per-core batch is 64 = half the PE output partitions; hidden units
    split in half across PSUM partition ranges (partitions 0:64 =
    (batch, units 0:512), 64:128 = (batch, units 512:1024)); the two
    column-groups of the PE run concurrently (verified in trace:
    216 ns/pair at N=512 bf16).
  * Gate bank order is (g, i, f, o) and the f bank runs as two
    half-width streams, so c = f*c + i*g and tanh(c) complete during the
    o bank's matmuls; only sigmoid(o)*tanh(c) + transpose + cast remain
    on the step-boundary critical path (split into column halves, with
    k-chunk order 0,4,1,5,... so the next step starts after half a cast).
  * X-injection matmuls for step t+1 issue right after step t's pairs,
    filling the PE during the epilogue.
  * h stays bf16 end-to-end: bf16 PE transposes into a bf16 PSUM tile,
    one 2x-rate DVE cast back to SBUF.
"""

import os
import numpy as np

B, T = 512, 128
HIDDEN = 1024
VOCAB = 1024
EMB = 512
OUT = 1024
D_IMG = 2048
NCORES = 8
BS = B // NCORES  # 64 batch rows per core
HH = HIDDEN // 2  # 512 = per-half hidden units

_CACHE = {}

# gate order inside the packed tables / PSUM banks: (g, i, f, o)
GATE_PERM = [1, 0, 2, 3]  # indices into natural (i, g, f, o)
BANK_G, BANK_I, BANK_F, BANK_O = 0, 1, 2, 3
CI_ORDER = [0, 4, 1, 5, 2, 6, 3, 7]  # k-chunks: first 4 need tp blocks 0,1


def _build_nc(n_steps: int):
    import concourse.bass as bass
    import concourse.mybir as mybir
    from concourse import bacc, tile

    f32 = mybir.dt.float32
    f32r = mybir.dt.float32r
    bf16 = mybir.dt.bfloat16
    i32 = mybir.dt.int32
    AF = mybir.ActivationFunctionType

    nc = bacc.Bacc("TRN2", target_bir_lowering=False, debug=False)

    m2p_d = nc.declare_dram_parameter("m2p", [2 * VOCAB, HH * 4], bf16, isOutput=False)
    wh_d = nc.declare_dram_parameter("wh", [HIDDEN, 4 * HIDDEN], bf16, isOutput=False)
    msg2_d = nc.declare_dram_parameter("msg2", [2 * BS, T], i32, isOutput=False)
    sfull_d = nc.declare_dram_parameter("sfull", [2 * BS, 2 * BS], bf16, isOutput=False)
    ident_d = nc.declare_dram_parameter("ident", [128, 128], f32, isOutput=False)
    identb_d = nc.declare_dram_parameter("identb", [128, 128], bf16, isOutput=False)
    imgs_d = nc.declare_dram_parameter("imgs", [128, D_IMG // 2], f32, isOutput=False)
    wimg_d = nc.declare_dram_parameter("wimg", [D_IMG, OUT], bf16, isOutput=False)
    whid_d = nc.declare_dram_parameter("whid", [HIDDEN, OUT], bf16, isOutput=False)
    o2_d = nc.declare_dram_parameter("o2", [2, 128], f32r, isOutput=False)
    bimg2_d = nc.declare_dram_parameter("bimg2", [2, OUT // 2], f32r, isOutput=False)
    bhid2_d = nc.declare_dram_parameter("bhid2", [2, OUT // 2], f32r, isOutput=False)
    oimg_d = nc.declare_dram_parameter("oimg", [128, OUT // 2], f32, isOutput=True)
    ohid_d = nc.declare_dram_parameter("ohid", [128, OUT // 2], f32, isOutput=True)

    with tile.TileContext(nc) as tc:
        with (
            tc.tile_pool(name="wpool", bufs=1) as wpool,
            tc.tile_pool(name="const", bufs=1) as cpool,
            tc.tile_pool(name="xg", bufs=3) as xgpool,
            tc.tile_pool(name="state", bufs=2) as stpool,
            tc.tile_pool(name="act", bufs=1) as apool,
            tc.tile_pool(name="outs", bufs=1) as opool,
            tc.tile_pool(name="psum", bufs=1, space="PSUM") as pspool,
        ):
            # ---- constants / small inputs ----
            msg2 = cpool.tile([2 * BS, T], i32, tag="msg2")
            nc.sync.dma_start(msg2[:], msg2_d[:])
            sfull = cpool.tile([2 * BS, 2 * BS], bf16, tag="sfull")
            nc.sync.dma_start(sfull[:], sfull_d[:])
            ident = cpool.tile([128, 128], f32, tag="ident")
            nc.sync.dma_start(ident[:], ident_d[:])
            identb = cpool.tile([128, 128], bf16, tag="identb")
            nc.sync.dma_start(identb[:], identb_d[:])
            o2 = cpool.tile([2, 128], f32r, tag="o2")
            nc.sync.dma_start(o2[:], o2_d[:])
            bimg2 = cpool.tile([2, OUT // 2], f32r, tag="bimg2")
            nc.sync.dma_start(bimg2[:], bimg2_d[:])
            bhid2 = cpool.tile([2, OUT // 2], f32r, tag="bhid2")
            nc.sync.dma_start(bhid2[:], bhid2_d[:])

            # ---- W_h resident in SBUF: 8 chunks of [128, 4096] ----
            wh_sb = []
            for ci in range(8):
                wt = wpool.tile([128, 4 * HIDDEN], bf16, tag=f"wh{ci}")
                nc.sync.dma_start(wt[:], wh_d[128 * ci : 128 * (ci + 1), :])
                wh_sb.append(wt)
            # encoder weights resident too (DMA overlaps the recurrence)
            whid_sb = []
            for ci in range(8):
                wt = wpool.tile([128, OUT], bf16, tag=f"whid{ci}")
                nc.sync.dma_start(wt[:], whid_d[128 * ci : 128 * (ci + 1), :])
                whid_sb.append(wt)
            wimg_sb = []
            for ci in range(16):
                wt = wpool.tile([128, OUT], bf16, tag=f"wimg{ci}")
                nc.sync.dma_start(wt[:], wimg_d[128 * ci : 128 * (ci + 1), :])
                wimg_sb.append(wt)

            # ---- images transposed up front (also PE warmup) ----
            # imgs packed [128, 1024]: partitions 0:64 = batch x feats 0:1024,
            # partitions 64:128 = batch x feats 1024:2048. Cast to bf16 first
            # so the transposes share the bf16 "tp" PSUM tile with the loop.
            imgs = cpool.tile([128, D_IMG // 2], f32, tag="imgs")
            nc.sync.dma_start(imgs[:], imgs_d[:])
            imgsb = cpool.tile([128, D_IMG // 2], bf16, tag="imgsb")
            nc.vector.tensor_copy(imgsb[:], imgs[:])
            imT = cpool.tile([128, D_IMG // 2], bf16, tag="imT")
            for half in range(2):
                tpw = pspool.tile([128, 8 * BS], bf16, tag="tp")
                for q in range(4):
                    qq = 4 * half + q
                    nc.tensor.transpose(
                        out=tpw[:, 128 * q : 128 * (q + 1)],
                        in_=imgsb[:, 128 * qq : 128 * (qq + 1)],
                        identity=identb[:],
                    )
                nc.vector.tensor_copy(imT[:, 512 * half : 512 * (half + 1)], tpw[:])

            # ---- LSTM state init ----
            c_prev = stpool.tile([128, HH], f32, tag="c")
            nc.vector.memset(c_prev[:], 0.0)
            hT_cur = None

            def hT_sl(hT, ci):
                # packed-transpose layout: pair q holds chunk q (cols 0:64)
                # and chunk q+4 (cols 64:128) at col block 128*q
                q, hi = (ci - 4, 64) if ci >= 4 else (ci, 0)
                return hT[:, 128 * q + hi : 128 * q + hi + 64]

            def imT_sl(ci):
                q, hi = (ci - 8, 64) if ci >= 8 else (ci, 0)
                return imT[:, 128 * q + hi : 128 * q + hi + 64]

            def gather(t):
                xg = xgpool.tile([2 * BS, 4 * HH], bf16, tag="xg", name=f"xg_{t}")
                nc.gpsimd.indirect_dma_start(
                    out=xg[:],
                    out_offset=None,
                    in_=m2p_d[:],
                    in_offset=bass.IndirectOffsetOnAxis(ap=msg2[:, t : t + 1], axis=0),
                )
                return xg

            # Gate PSUM tiles: g and i full banks; f and o split into two
            # half-written full-bank tiles each, so a half-stream's consumer
            # only depends on that half's matmuls (deps are tile-granular) and
            # PE writes never share a bank with ACT/DVE reads.
            # STREAMS: (key, gate, gate_c0, gate_c1, out_c0)
            STREAMS = (
                ("g", 0, 0, HH, 0),
                ("i", 1, 0, HH, 0),
                ("f1", 2, 0, HH // 2, 0),
                ("f2", 2, HH // 2, HH, 0),
                ("o1", 3, 0, HH // 2, 0),
                ("o2", 3, HH // 2, HH, 0),
            )

            def alloc_banks(t):
                return {
                    k: pspool.tile([128, HH], f32, tag=f"gp_{k}", name=f"gp_{k}_{t}")
                    for k in ("g", "i", "f1", "f2", "o1", "o2")
                }

            X_FLOOR = {"g": 0.0084, "i": 0.0086, "f1": 0.0088, "f2": 0.009,
                       "o1": 0.0092}

            def x_inject(gpb, xg, t, stop, base=None):
                # seeds every bank (start=True over all 128 partitions);
                # f/o injects are sim-floored past transp01 so they don't
                # queue-block the transposes -> cast1 -> first-pair chain
                for k, gate, c0, c1, oc0 in STREAMS:
                    fl = X_FLOOR.get(k) if base is not None else None
                    with tc.tile_wait_until(0 if fl is None else base + fl,
                                            enable=fl is not None):
                        nc.tensor.matmul(
                            out=gpb[k][:, oc0 : oc0 + (c1 - c0)],
                            lhsT=sfull[:],
                            rhs=xg[:, HH * gate + c0 : HH * gate + c1],
                            start=True,
                            stop=stop,
                            skip_group_check=True,
                        )

            # ---- prologue: gathers + step-0 gates (pure x contribution) ----
            xg_t = [None] * (n_steps + 2)
            xg_t[0] = gather(0)
            xg_t[1] = gather(1)
            gpb_cur = alloc_banks(0)
            x_inject(gpb_cur, xg_t[0], 0, stop=True)

            # ---- recurrence ----
            for t in range(n_steps):
                last_t = t == n_steps - 1
                if t + 2 <= n_steps - 1:
                    xg_t[t + 2] = gather(t + 2)

                gpb = gpb_cur
                if t > 0:
                    # h @ W_h pairs: stream order (g, i, f1, f2, o1, o2) so
                    # the c chain finishes during the o streams. Sim-floored
                    # past the previous tail's cast1 so the scheduler emits
                    # the tail (transposes/casts/X) ahead of the stream.
                    ctx_pairs = tc.tile_wait_until(0.022 * t + 0.009)
                    ctx_pairs.__enter__()
                    for k, gate, c0, c1, oc0 in STREAMS:
                        w0 = 1024 * gate
                        for ci in CI_ORDER:
                            last = ci == CI_ORDER[-1]
                            lhs = hT_sl(hT_cur, ci)
                            nc.tensor.matmul(
                                out=gpb[k][0:64, oc0 : oc0 + (c1 - c0)],
                                lhsT=lhs,
                                rhs=wh_sb[ci][:, w0 + c0 : w0 + c1],
                                start=False,
                                stop=last,
                                skip_group_check=True,
                            )
                            nc.tensor.matmul(
                                out=gpb[k][64:128, oc0 : oc0 + (c1 - c0)],
                                lhsT=lhs,
                                rhs=wh_sb[ci][:, w0 + 512 + c0 : w0 + 512 + c1],
                                start=False,
                                stop=last,
                                skip_group_check=True,
                            )

                if t > 0:
                    ctx_pairs.__exit__(None, None, None)

                # X injection for step t+1 (fills the PE during the epilogue;
                # each bank's inject waits only for its step-t activation)
                if not last_t:
                    gpb_next = alloc_banks(t + 1)
                    x_inject(gpb_next, xg_t[t + 1], t + 1, stop=False,
                             base=0.022 * (t + 1))

                # ---- epilogue: gates -> c, h ----
                # ACT issue order is forced with sim-dispatch floors on a
                # 22 us grid (scheduling-pass only, no runtime cost): the
                # scheduler's cost model misjudges the PE stream (it doesn't
                # model col-group concurrency) and would otherwise hoist the
                # data-chained tanh ops ahead of the stop-gated sigmoids.
                base = 0.022 * (t + 1)
                tg = apool.tile([128, HH], f32, tag="tg")
                with tc.tile_wait_until(base):
                    nc.scalar.activation(tg[:], gpb["g"][:], AF.Tanh)
                si = apool.tile([128, HH], f32, tag="si")
                with tc.tile_wait_until(base + 0.0015):
                    nc.scalar.activation(si[:], gpb["i"][:], AF.Sigmoid)
                m1 = apool.tile([128, HH], f32, tag="m1")
                nc.vector.tensor_mul(m1[:], si[:], tg[:])
                sf = apool.tile([128, HH], f32, tag="sf")
                c_new = stpool.tile([128, HH], f32, tag="c")
                tc_ = apool.tile([128, HH], bf16, tag="tc")
                so = apool.tile([128, HH], bf16, tag="so")
                h = apool.tile([128, HH], bf16, tag="h")
                cm = apool.tile([128, HH], f32, tag="cm")
                HQ = HH // 2
                for hf in range(2):
                    sl = slice(HQ * hf, HQ * (hf + 1))
                    with tc.tile_wait_until(base + 0.003 + 0.0015 * hf):
                        nc.scalar.activation(
                            sf[:, sl], gpb[f"f{hf + 1}"][:, 0:HQ], AF.Sigmoid
                        )
                    nc.vector.tensor_mul(cm[:, sl], sf[:, sl], c_prev[:, sl])
                    nc.vector.tensor_add(c_new[:, sl], cm[:, sl], m1[:, sl])
                # interleave tanh(c) and sigmoid(o) so the h blocks pipeline;
                # the o1 half runs as two quarter-ops so h block 0 (and its
                # transpose + cast, which gate the next stream) finishes
                # right after the o1 streams stop.
                with tc.tile_wait_until(base + 0.006):
                    nc.scalar.activation(tc_[:, 0:HQ], c_new[:, 0:HQ], AF.Tanh)
                for q in range(2):
                    qs = slice(128 * q, 128 * (q + 1))
                    with tc.tile_wait_until(base + 0.0075 + 0.0008 * q):
                        nc.scalar.activation(
                            so[:, qs], gpb["o1"][:, qs], AF.Sigmoid
                        )
                    nc.vector.tensor_mul(h[:, qs], so[:, qs], tc_[:, qs])
                with tc.tile_wait_until(base + 0.009):
                    nc.scalar.activation(tc_[:, HQ:HH], c_new[:, HQ:HH], AF.Tanh)
                with tc.tile_wait_until(base + 0.0105):
                    nc.scalar.activation(
                        so[:, HQ:HH], gpb["o2"][:, 0:HQ], AF.Sigmoid
                    )
                nc.vector.tensor_mul(h[:, HQ:HH], so[:, HQ:HH], tc_[:, HQ:HH])

                # h -> h^T: 4 bf16 PE transposes + 2 half casts; the next
                # step's first 4 pairs (chunks 0,4,1,5) need only cast half 1.
                tp = pspool.tile([128, 8 * BS], bf16, tag="tp")
                hT_next = stpool.tile([128, 8 * BS], bf16, tag="hT")
                for half in range(2):
                    for q in (2 * half, 2 * half + 1):
                        nc.tensor.transpose(
                            out=tp[:, 128 * q : 128 * (q + 1)],
                            in_=h[:, 128 * q : 128 * (q + 1)],
                            identity=identb[:],
                        )
                    if half == 0:
                        nc.vector.tensor_copy(hT_next[:, 0:128], tp[:, 0:128])
                        nc.vector.tensor_copy(hT_next[:, 128:256], tp[:, 128:256])
                    else:
                        nc.vector.tensor_copy(hT_next[:, 256:512], tp[:, 256:512])

                c_prev = c_new
                hT_cur = hT_next
                if not last_t:
                    gpb_cur = gpb_next

            # ---- hidden encoder: out = h @ W_hid + b_hid ----
            ohp = pspool.tile([128, OUT // 2], f32, tag="gp_g")
            nc.tensor.matmul(
                out=ohp[:], lhsT=o2[:], rhs=bhid2[:],
                start=True, stop=False, skip_group_check=True,
            )
            for ci in range(8):
                last = ci == 7
                lhs = hT_sl(hT_cur, ci)
                nc.tensor.matmul(
                    out=ohp[0:64, :], lhsT=lhs, rhs=whid_sb[ci][:, 0:512],
                    start=False, stop=last, skip_group_check=True,
                )
                nc.tensor.matmul(
                    out=ohp[64:128, :], lhsT=lhs, rhs=whid_sb[ci][:, 512:1024],
                    start=False, stop=last, skip_group_check=True,
                )
            ohid_sb = opool.tile([128, OUT // 2], f32, tag="ohid")
            nc.vector.tensor_copy(ohid_sb[:], ohp[:])
            nc.sync.dma_start(ohid_d[:], ohid_sb[:])

            # ---- images encoder: out = images @ W_img + b_img ----
            oip = pspool.tile([128, OUT // 2], f32, tag="gp_i")
            nc.tensor.matmul(
                out=oip[:], lhsT=o2[:], rhs=bimg2[:],
                start=True, stop=False, skip_group_check=True,
            )
            for ci in range(16):
                last = ci == 15
                lhs = imT_sl(ci)
                nc.tensor.matmul(
                    out=oip[0:64, :], lhsT=lhs, rhs=wimg_sb[ci][:, 0:512],
                    start=False, stop=last, skip_group_check=True,
                )
                nc.tensor.matmul(
                    out=oip[64:128, :], lhsT=lhs, rhs=wimg_sb[ci][:, 512:1024],
                    start=False, stop=last, skip_group_check=True,
                )
            oimg_sb = opool.tile([128, OUT // 2], f32, tag="oimg")
            nc.vector.tensor_copy(oimg_sb[:], oip[:])
            nc.sync.dma_start(oimg_d[:], oimg_sb[:])

    nc.compile()
    return nc


def _host_prep(images, embed_table, W_cell, b_cell, W_img, b_img, W_hid, b_hid,
               message):
    """Builds the per-core input maps (all host-side preprocessing)."""
    from ml_dtypes import bfloat16

    W_x = W_cell[:EMB]          # [512, 4096]
    W_h = W_cell[EMB:]          # [1024, 4096]

    M2 = embed_table.astype(np.float32) @ W_x + b_cell  # [1024, 4096]
    M2[:, 2 * HIDDEN : 3 * HIDDEN] += 1.0  # fold the f-gate +1.0
    # permute gate blocks to bank order (g, i, f, o)
    M2 = np.concatenate(
        [M2[:, 1024 * p : 1024 * (p + 1)] for p in GATE_PERM], axis=1
    )
    W_hp = np.concatenate(
        [W_h[:, 1024 * p : 1024 * (p + 1)] for p in GATE_PERM], axis=1
    ).astype(bfloat16)
    # row 2v+h = [g_h, i_h, f_h, o_h] halves of vocab row v
    M2p = np.ascontiguousarray(
        M2.reshape(VOCAB, 4, 2, HH).transpose(0, 2, 1, 3).reshape(2 * VOCAB, 4 * HH)
    ).astype(bfloat16)

    sfull = np.zeros((2 * BS, 2 * BS), np.float32)
    for m in range(BS):
        sfull[2 * m, m] = 1.0
        sfull[2 * m + 1, BS + m] = 1.0
    sfull = sfull.astype(bfloat16)

    ident = np.eye(128, dtype=np.float32)
    identb = ident.astype(bfloat16)

    o2 = np.zeros((2, 128), np.float32)
    o2[0, 0:64] = 1.0
    o2[1, 64:128] = 1.0

    W_img_b = W_img.astype(bfloat16)
    W_hid_b = W_hid.astype(bfloat16)
    bimg2 = np.stack([b_img[: OUT // 2], b_img[OUT // 2 :]]).astype(np.float32)
    bhid2 = np.stack([b_hid[: OUT // 2], b_hid[OUT // 2 :]]).astype(np.float32)

    in_maps = []
    for core in range(NCORES):
        sl = slice(core * BS, (core + 1) * BS)
        msg = message[sl]  # [64, T] int32
        msg2 = np.empty((2 * BS, T), np.int32)
        msg2[0::2] = 2 * msg
        msg2[1::2] = 2 * msg + 1
        in_maps.append(
            {
                "m2p": M2p,
                "wh": W_hp,
                "msg2": msg2,
                "sfull": sfull,
                "ident": ident,
                "identb": identb,
                "imgs": np.concatenate(
                    [images[sl, : D_IMG // 2], images[sl, D_IMG // 2 :]], axis=0
                ),
                "wimg": W_img_b,
                "whid": W_hid_b,
                "o2": o2,
                "bimg2": bimg2,
                "bhid2": bhid2,
            }
        )
    return in_maps


def kernel(images, embed_table, W_cell, b_cell, W_img, b_img, W_hid, b_hid,
           message):
    import sys
    if "/opt/trn_rl_repo" not in sys.path:
        sys.path.insert(0, "/opt/trn_rl_repo")
    from concourse.bass_utils import run_bass_kernel_spmd

    images = np.asarray(images, np.float32)
    embed_table = np.asarray(embed_table, np.float32)
    W_cell = np.asarray(W_cell, np.float32)
    b_cell = np.asarray(b_cell, np.float32)
    W_img = np.asarray(W_img, np.float32)
    b_img = np.asarray(b_img, np.float32)
    W_hid = np.asarray(W_hid, np.float32)
    b_hid = np.asarray(b_hid, np.float32)
    message = np.asarray(message, np.int32)

    n_steps = T
    if "nc" not in _CACHE or _CACHE.get("n_steps") != n_steps:
        _CACHE["nc"] = _build_nc(n_steps)
        _CACHE["n_steps"] = n_steps
    nc = _CACHE["nc"]

    in_maps = _host_prep(
        images, embed_table, W_cell, b_cell, W_img, b_img, W_hid, b_hid, message
    )
    res = run_bass_kernel_spmd(nc, in_maps, core_ids=list(range(NCORES)))
    results = res.results

    images_encoded = np.empty((B, OUT), np.float32)
    hidden_encoded = np.empty((B, OUT), np.float32)
    for core in range(NCORES):
        sl = slice(core * BS, (core + 1) * BS)
        oi = results[core]["oimg"]
        oh = results[core]["ohid"]
        images_encoded[sl, : OUT // 2] = oi[0:64]
        images_encoded[sl, OUT // 2 :] = oi[64:128]
        hidden_encoded[sl, : OUT // 2] = oh[0:64]
        hidden_encoded[sl, OUT // 2 :] = oh[64:128]
    return images_encoded, hidden_encoded


# revision 17
# speedup vs baseline: 1.0144x; 1.0144x over previous
"""Trainium2 Bass kernel for nn_Listener (LSTM listener + dense encoders).

Reference computation (per full batch B=512):
    emb = embed_table[message]                       # [B, T, 512]
    LSTM over T=128 steps, HIDDEN=1024:
        gated = [x_t, h] @ W_cell + b_cell           # [B, 4096] (i, g, f, o)
        f = sigmoid(f + 1); c = f*c + sigmoid(i)*tanh(g); h = sigmoid(o)*tanh(c)
    images_encoded = images @ W_img + b_img          # [B, 1024]
    hidden_encoded = h @ W_hid + b_hid               # [B, 1024]
    returns (images_encoded, hidden_encoded)

Strategy (8 NeuronCores, data-parallel over batch, 64 rows/core):
  * Embedding lookup + x-projection fold into one table:
        M2 = embed_table @ W_cell[:512] + b_cell  (+1 on the f columns)
    stored bf16; per step the rows are gathered by token id and injected
    into PSUM with a k=128 bf16 selection matmul that also seeds the
    accumulation groups.
  * Per-core batch is 64 = half the PE output partitions; hidden units
    split in half across PSUM partition ranges (partitions 0:64 =
    (batch, units 0:512), 64:128 = (batch, units 512:1024)); the two
    column-groups of the PE run concurrently (verified in trace:
    216 ns/pair at N=512 bf16).
  * Gate bank order is (g, i, f, o) and the f bank runs as two
    half-width streams, so c = f*c + i*g and tanh(c) complete during the
    o bank's matmuls; only sigmoid(o)*tanh(c) + transpose + cast remain
    on the step-boundary critical path (split into column halves, with
    k-chunk order 0,4,1,5,... so the next step starts after half a cast).
  * X-injection matmuls for step t+1 issue right after step t's pairs,
    filling the PE during the epilogue.
  * h stays bf16 end-to-end: bf16 PE transposes into a bf16 PSUM tile,
    one 2x-rate DVE cast back to SBUF.
"""

import os
import numpy as np

B, T = 512, 128
HIDDEN = 1024
VOCAB = 1024
EMB = 512
OUT = 1024
D_IMG = 2048
NCORES = 8
BS = B // NCORES  # 64 batch rows per core
HH = HIDDEN // 2  # 512 = per-half hidden units

_CACHE = {}

# gate order inside the packed tables / PSUM banks: (g, i, f, o)
GATE_PERM = [1, 0, 2, 3]  # indices into natural (i, g, f, o)
BANK_G, BANK_I, BANK_F, BANK_O = 0, 1, 2, 3
CI_ORDER = [0, 4, 1, 5, 2, 6, 3, 7]  # k-chunks: first 4 need tp blocks 0,1


def _build_nc(n_steps: int):
    import concourse.bass as bass
    import concourse.mybir as mybir
    from concourse import bacc, tile

    f32 = mybir.dt.float32
    f32r = mybir.dt.float32r
    bf16 = mybir.dt.bfloat16
    i32 = mybir.dt.int32
    AF = mybir.ActivationFunctionType

    nc = bacc.Bacc("TRN2", target_bir_lowering=False, debug=False)

    m2p_d = nc.declare_dram_parameter("m2p", [2 * VOCAB, HH * 4], bf16, isOutput=False)
    wh_d = nc.declare_dram_parameter("wh", [HIDDEN, 4 * HIDDEN], bf16, isOutput=False)
    msg2_d = nc.declare_dram_parameter("msg2", [2 * BS, T], i32, isOutput=False)
    sfull_d = nc.declare_dram_parameter("sfull", [2 * BS, 2 * BS], bf16, isOutput=False)
    ident_d = nc.declare_dram_parameter("ident", [128, 128], f32, isOutput=False)
    identb_d = nc.declare_dram_parameter("identb", [128, 128], bf16, isOutput=False)
    imgs_d = nc.declare_dram_parameter("imgs", [128, D_IMG // 2], f32, isOutput=False)
    wimg_d = nc.declare_dram_parameter("wimg", [D_IMG, OUT], bf16, isOutput=False)
    whid_d = nc.declare_dram_parameter("whid", [HIDDEN, OUT], bf16, isOutput=False)
    o2_d = nc.declare_dram_parameter("o2", [2, 128], f32r, isOutput=False)
    bimg2_d = nc.declare_dram_parameter("bimg2", [2, OUT // 2], f32r, isOutput=False)
    bhid2_d = nc.declare_dram_parameter("bhid2", [2, OUT // 2], f32r, isOutput=False)
    oimg_d = nc.declare_dram_parameter("oimg", [128, OUT // 2], f32, isOutput=True)
    ohid_d = nc.declare_dram_parameter("ohid", [128, OUT // 2], f32, isOutput=True)

    with tile.TileContext(nc) as tc:
        with (
            tc.tile_pool(name="wpool", bufs=1) as wpool,
            tc.tile_pool(name="const", bufs=1) as cpool,
            tc.tile_pool(name="xg", bufs=3) as xgpool,
            tc.tile_pool(name="state", bufs=2) as stpool,
            tc.tile_pool(name="act", bufs=1) as apool,
            tc.tile_pool(name="outs", bufs=1) as opool,
            tc.tile_pool(name="psum", bufs=1, space="PSUM") as pspool,
        ):
            # ---- constants / small inputs ----
            msg2 = cpool.tile([2 * BS, T], i32, tag="msg2")
            nc.sync.dma_start(msg2[:], msg2_d[:])
            sfull = cpool.tile([2 * BS, 2 * BS], bf16, tag="sfull")
            nc.sync.dma_start(sfull[:], sfull_d[:])
            ident = cpool.tile([128, 128], f32, tag="ident")
            nc.sync.dma_start(ident[:], ident_d[:])
            identb = cpool.tile([128, 128], bf16, tag="identb")
            nc.sync.dma_start(identb[:], identb_d[:])
            o2 = cpool.tile([2, 128], f32r, tag="o2")
            nc.sync.dma_start(o2[:], o2_d[:])
            bimg2 = cpool.tile([2, OUT // 2], f32r, tag="bimg2")
            nc.sync.dma_start(bimg2[:], bimg2_d[:])
            bhid2 = cpool.tile([2, OUT // 2], f32r, tag="bhid2")
            nc.sync.dma_start(bhid2[:], bhid2_d[:])

            # ---- W_h resident in SBUF: 8 chunks of [128, 4096] ----
            wh_sb = []
            for ci in range(8):
                wt = wpool.tile([128, 4 * HIDDEN], bf16, tag=f"wh{ci}")
                nc.sync.dma_start(wt[:], wh_d[128 * ci : 128 * (ci + 1), :])
                wh_sb.append(wt)
            # encoder weights resident too (DMA overlaps the recurrence)
            whid_sb = []
            for ci in range(8):
                wt = wpool.tile([128, OUT], bf16, tag=f"whid{ci}")
                nc.sync.dma_start(wt[:], whid_d[128 * ci : 128 * (ci + 1), :])
                whid_sb.append(wt)
            wimg_sb = []
            for ci in range(16):
                wt = wpool.tile([128, OUT], bf16, tag=f"wimg{ci}")
                nc.sync.dma_start(wt[:], wimg_d[128 * ci : 128 * (ci + 1), :])
                wimg_sb.append(wt)

            # ---- images transposed up front (also PE warmup) ----
            # imgs packed [128, 1024]: partitions 0:64 = batch x feats 0:1024,
            # partitions 64:128 = batch x feats 1024:2048. Cast to bf16 first
            # so the transposes share the bf16 "tp" PSUM tile with the loop.
            imgs = cpool.tile([128, D_IMG // 2], f32, tag="imgs")
            nc.sync.dma_start(imgs[:], imgs_d[:])
            imgsb = cpool.tile([128, D_IMG // 2], bf16, tag="imgsb")
            nc.vector.tensor_copy(imgsb[:], imgs[:])
            imT = cpool.tile([128, D_IMG // 2], bf16, tag="imT")
            for half in range(2):
                tpw = pspool.tile([128, 8 * BS], bf16, tag="tp")
                for q in range(4):
                    qq = 4 * half + q
                    nc.tensor.transpose(
                        out=tpw[:, 128 * q : 128 * (q + 1)],
                        in_=imgsb[:, 128 * qq : 128 * (qq + 1)],
                        identity=identb[:],
                    )
                nc.vector.tensor_copy(imT[:, 512 * half : 512 * (half + 1)], tpw[:])

            # ---- LSTM state init ----
            c_prev = stpool.tile([128, HH], f32, tag="c")
            nc.vector.memset(c_prev[:], 0.0)
            hT_cur = None

            def hT_sl(hT, ci):
                # packed-transpose layout: pair q holds chunk q (cols 0:64)
                # and chunk q+4 (cols 64:128) at col block 128*q
                q, hi = (ci - 4, 64) if ci >= 4 else (ci, 0)
                return hT[:, 128 * q + hi : 128 * q + hi + 64]

            def imT_sl(ci):
                q, hi = (ci - 8, 64) if ci >= 8 else (ci, 0)
                return imT[:, 128 * q + hi : 128 * q + hi + 64]

            def gather(t):
                xg = xgpool.tile([2 * BS, 4 * HH], bf16, tag="xg", name=f"xg_{t}")
                nc.gpsimd.indirect_dma_start(
                    out=xg[:],
                    out_offset=None,
                    in_=m2p_d[:],
                    in_offset=bass.IndirectOffsetOnAxis(ap=msg2[:, t : t + 1], axis=0),
                )
                return xg

            # Gate PSUM tiles: g and i full banks; f and o split into two
            # half-written full-bank tiles each, so a half-stream's consumer
            # only depends on that half's matmuls (deps are tile-granular) and
            # PE writes never share a bank with ACT/DVE reads.
            # STREAMS: (key, gate, gate_c0, gate_c1, out_c0)
            STREAMS = (
                ("g", 0, 0, HH, 0),
                ("i", 1, 0, HH, 0),
                ("f1", 2, 0, HH // 2, 0),
                ("f2", 2, HH // 2, HH, 0),
                ("o1", 3, 0, HH // 2, 0),
                ("o2", 3, HH // 2, HH, 0),
            )

            def alloc_banks(t):
                return {
                    k: pspool.tile([128, HH], f32, tag=f"gp_{k}", name=f"gp_{k}_{t}")
                    for k in ("g", "i", "f1", "f2", "o1", "o2")
                }

            X_FLOOR = {"i": 0.0086, "f1": 0.0088, "f2": 0.009, "o1": 0.0092}

            def x_inject(gpb, xg, t, stop, base=None):
                # seeds every bank (start=True over all 128 partitions);
                # f/o injects are sim-floored past transp01 so they don't
                # queue-block the transposes -> cast1 -> first-pair chain
                for k, gate, c0, c1, oc0 in STREAMS:
                    fl = X_FLOOR.get(k) if base is not None else None
                    with tc.tile_wait_until(0 if fl is None else base + fl,
                                            enable=fl is not None):
                        nc.tensor.matmul(
                            out=gpb[k][:, oc0 : oc0 + (c1 - c0)],
                            lhsT=sfull[:],
                            rhs=xg[:, HH * gate + c0 : HH * gate + c1],
                            start=True,
                            stop=stop,
                            skip_group_check=True,
                        )

            # ---- prologue: gathers + step-0 gates (pure x contribution) ----
            xg_t = [None] * (n_steps + 2)
            xg_t[0] = gather(0)
            xg_t[1] = gather(1)
            gpb_cur = alloc_banks(0)
            x_inject(gpb_cur, xg_t[0], 0, stop=True)

            # ---- recurrence ----
            for t in range(n_steps):
                last_t = t == n_steps - 1
                if t + 2 <= n_steps - 1:
                    xg_t[t + 2] = gather(t + 2)

                gpb = gpb_cur
                if t > 0:
                    # h @ W_h pairs: stream order (g, i, f1, f2, o1, o2) so
                    # the c chain finishes during the o streams. Sim-floored
                    # past the previous tail's cast1 so the scheduler emits
                    # the tail (transposes/casts/X) ahead of the stream.
                    ctx_pairs = tc.tile_wait_until(0.022 * t + 0.009)
                    ctx_pairs.__enter__()
                    for k, gate, c0, c1, oc0 in STREAMS:
                        w0 = 1024 * gate
                        for ci in CI_ORDER:
                            last = ci == CI_ORDER[-1]
                            lhs = hT_sl(hT_cur, ci)
                            nc.tensor.matmul(
                                out=gpb[k][0:64, oc0 : oc0 + (c1 - c0)],
                                lhsT=lhs,
                                rhs=wh_sb[ci][:, w0 + c0 : w0 + c1],
                                start=False,
                                stop=last,
                                skip_group_check=True,
                            )
                            nc.tensor.matmul(
                                out=gpb[k][64:128, oc0 : oc0 + (c1 - c0)],
                                lhsT=lhs,
                                rhs=wh_sb[ci][:, w0 + 512 + c0 : w0 + 512 + c1],
                                start=False,
                                stop=last,
                                skip_group_check=True,
                            )

                if t > 0:
                    ctx_pairs.__exit__(None, None, None)

                # X injection for step t+1 (fills the PE during the epilogue;
                # each bank's inject waits only for its step-t activation)
                if not last_t:
                    gpb_next = alloc_banks(t + 1)
                    x_inject(gpb_next, xg_t[t + 1], t + 1, stop=False,
                             base=0.022 * (t + 1))

                # ---- epilogue: gates -> c, h ----
                # ACT issue order is forced with sim-dispatch floors on a
                # 22 us grid (scheduling-pass only, no runtime cost): the
                # scheduler's cost model misjudges the PE stream (it doesn't
                # model col-group concurrency) and would otherwise hoist the
                # data-chained tanh ops ahead of the stop-gated sigmoids.
                base = 0.022 * (t + 1)
                tg = apool.tile([128, HH], f32, tag="tg")
                with tc.tile_wait_until(base):
                    nc.scalar.activation(tg[:], gpb["g"][:], AF.Tanh)
                si = apool.tile([128, HH], f32, tag="si")
                with tc.tile_wait_until(base + 0.0015):
                    nc.scalar.activation(si[:], gpb["i"][:], AF.Sigmoid)
                m1 = apool.tile([128, HH], f32, tag="m1")
                nc.vector.tensor_mul(m1[:], si[:], tg[:])
                sf = apool.tile([128, HH], f32, tag="sf")
                c_new = stpool.tile([128, HH], f32, tag="c")
                tc_ = apool.tile([128, HH], bf16, tag="tc")
                so = apool.tile([128, HH], bf16, tag="so")
                h = apool.tile([128, HH], bf16, tag="h")
                cm = apool.tile([128, HH], f32, tag="cm")
                HQ = HH // 2
                for hf in range(2):
                    sl = slice(HQ * hf, HQ * (hf + 1))
                    with tc.tile_wait_until(base + 0.003 + 0.0015 * hf):
                        nc.scalar.activation(
                            sf[:, sl], gpb[f"f{hf + 1}"][:, 0:HQ], AF.Sigmoid
                        )
                    nc.vector.tensor_mul(cm[:, sl], sf[:, sl], c_prev[:, sl])
                    nc.vector.tensor_add(c_new[:, sl], cm[:, sl], m1[:, sl])
                # interleave tanh(c) and sigmoid(o) halves so the h halves
                # pipeline: tc1, so1, tc2, so2 on the ACT queue
                for hf in range(2):
                    sl = slice(HQ * hf, HQ * (hf + 1))
                    with tc.tile_wait_until(base + 0.006 + 0.003 * hf):
                        nc.scalar.activation(tc_[:, sl], c_new[:, sl], AF.Tanh)
                    with tc.tile_wait_until(base + 0.0075 + 0.003 * hf):
                        nc.scalar.activation(
                            so[:, sl], gpb[f"o{hf + 1}"][:, 0:HQ], AF.Sigmoid
                        )
                    nc.vector.tensor_mul(h[:, sl], so[:, sl], tc_[:, sl])

                # h -> h^T: 4 bf16 PE transposes + 2 half casts; the next
                # step's first 4 pairs (chunks 0,4,1,5) need only cast half 1.
                tp = pspool.tile([128, 8 * BS], bf16, tag="tp")
                hT_next = stpool.tile([128, 8 * BS], bf16, tag="hT")
                for half in range(2):
                    for q in (2 * half, 2 * half + 1):
                        nc.tensor.transpose(
                            out=tp[:, 128 * q : 128 * (q + 1)],
                            in_=h[:, 128 * q : 128 * (q + 1)],
                            identity=identb[:],
                        )
                    if half == 0:
                        nc.vector.tensor_copy(hT_next[:, 0:128], tp[:, 0:128])
                        nc.vector.tensor_copy(hT_next[:, 128:256], tp[:, 128:256])
                    else:
                        nc.vector.tensor_copy(hT_next[:, 256:512], tp[:, 256:512])

                c_prev = c_new
                hT_cur = hT_next
                if not last_t:
                    gpb_cur = gpb_next

            # ---- hidden encoder: out = h @ W_hid + b_hid ----
            ohp = pspool.tile([128, OUT // 2], f32, tag="gp_g")
            nc.tensor.matmul(
                out=ohp[:], lhsT=o2[:], rhs=bhid2[:],
                start=True, stop=False, skip_group_check=True,
            )
            for ci in range(8):
                last = ci == 7
                lhs = hT_sl(hT_cur, ci)
                nc.tensor.matmul(
                    out=ohp[0:64, :], lhsT=lhs, rhs=whid_sb[ci][:, 0:512],
                    start=False, stop=last, skip_group_check=True,
                )
                nc.tensor.matmul(
                    out=ohp[64:128, :], lhsT=lhs, rhs=whid_sb[ci][:, 512:1024],
                    start=False, stop=last, skip_group_check=True,
                )
            ohid_sb = opool.tile([128, OUT // 2], f32, tag="ohid")
            nc.vector.tensor_copy(ohid_sb[:], ohp[:])
            nc.sync.dma_start(ohid_d[:], ohid_sb[:])

            # ---- images encoder: out = images @ W_img + b_img ----
            oip = pspool.tile([128, OUT // 2], f32, tag="gp_i")
            nc.tensor.matmul(
                out=oip[:], lhsT=o2[:], rhs=bimg2[:],
                start=True, stop=False, skip_group_check=True,
            )
            for ci in range(16):
                last = ci == 15
                lhs = imT_sl(ci)
                nc.tensor.matmul(
                    out=oip[0:64, :], lhsT=lhs, rhs=wimg_sb[ci][:, 0:512],
                    start=False, stop=last, skip_group_check=True,
                )
                nc.tensor.matmul(
                    out=oip[64:128, :], lhsT=lhs, rhs=wimg_sb[ci][:, 512:1024],
                    start=False, stop=last, skip_group_check=True,
                )
            oimg_sb = opool.tile([128, OUT // 2], f32, tag="oimg")
            nc.vector.tensor_copy(oimg_sb[:], oip[:])
            nc.sync.dma_start(oimg_d[:], oimg_sb[:])

    nc.compile()
    return nc


def _host_prep(images, embed_table, W_cell, b_cell, W_img, b_img, W_hid, b_hid,
               message):
    """Builds the per-core input maps (all host-side preprocessing)."""
    from ml_dtypes import bfloat16

    W_x = W_cell[:EMB]          # [512, 4096]
    W_h = W_cell[EMB:]          # [1024, 4096]

    M2 = embed_table.astype(np.float32) @ W_x + b_cell  # [1024, 4096]
    M2[:, 2 * HIDDEN : 3 * HIDDEN] += 1.0  # fold the f-gate +1.0
    # permute gate blocks to bank order (g, i, f, o)
    M2 = np.concatenate(
        [M2[:, 1024 * p : 1024 * (p + 1)] for p in GATE_PERM], axis=1
    )
    W_hp = np.concatenate(
        [W_h[:, 1024 * p : 1024 * (p + 1)] for p in GATE_PERM], axis=1
    ).astype(bfloat16)
    # row 2v+h = [g_h, i_h, f_h, o_h] halves of vocab row v
    M2p = np.ascontiguousarray(
        M2.reshape(VOCAB, 4, 2, HH).transpose(0, 2, 1, 3).reshape(2 * VOCAB, 4 * HH)
    ).astype(bfloat16)

    sfull = np.zeros((2 * BS, 2 * BS), np.float32)
    for m in range(BS):
        sfull[2 * m, m] = 1.0
        sfull[2 * m + 1, BS + m] = 1.0
    sfull = sfull.astype(bfloat16)

    ident = np.eye(128, dtype=np.float32)
    identb = ident.astype(bfloat16)

    o2 = np.zeros((2, 128), np.float32)
    o2[0, 0:64] = 1.0
    o2[1, 64:128] = 1.0

    W_img_b = W_img.astype(bfloat16)
    W_hid_b = W_hid.astype(bfloat16)
    bimg2 = np.stack([b_img[: OUT // 2], b_img[OUT // 2 :]]).astype(np.float32)
    bhid2 = np.stack([b_hid[: OUT // 2], b_hid[OUT // 2 :]]).astype(np.float32)

    in_maps = []
    for core in range(NCORES):
        sl = slice(core * BS, (core + 1) * BS)
        msg = message[sl]  # [64, T] int32
        msg2 = np.empty((2 * BS, T), np.int32)
        msg2[0::2] = 2 * msg
        msg2[1::2] = 2 * msg + 1
        in_maps.append(
            {
                "m2p": M2p,
                "wh": W_hp,
                "msg2": msg2,
                "sfull": sfull,
                "ident": ident,
                "identb": identb,
                "imgs": np.concatenate(
                    [images[sl, : D_IMG // 2], images[sl, D_IMG // 2 :]], axis=0
                ),
                "wimg": W_img_b,
                "whid": W_hid_b,
                "o2": o2,
                "bimg2": bimg2,
                "bhid2": bhid2,
            }
        )
    return in_maps


def kernel(images, embed_table, W_cell, b_cell, W_img, b_img, W_hid, b_hid,
           message):
    import sys
    if "/opt/trn_rl_repo" not in sys.path:
        sys.path.insert(0, "/opt/trn_rl_repo")
    from concourse.bass_utils import run_bass_kernel_spmd

    images = np.asarray(images, np.float32)
    embed_table = np.asarray(embed_table, np.float32)
    W_cell = np.asarray(W_cell, np.float32)
    b_cell = np.asarray(b_cell, np.float32)
    W_img = np.asarray(W_img, np.float32)
    b_img = np.asarray(b_img, np.float32)
    W_hid = np.asarray(W_hid, np.float32)
    b_hid = np.asarray(b_hid, np.float32)
    message = np.asarray(message, np.int32)

    n_steps = T
    if "nc" not in _CACHE or _CACHE.get("n_steps") != n_steps:
        _CACHE["nc"] = _build_nc(n_steps)
        _CACHE["n_steps"] = n_steps
    nc = _CACHE["nc"]

    in_maps = _host_prep(
        images, embed_table, W_cell, b_cell, W_img, b_img, W_hid, b_hid, message
    )
    res = run_bass_kernel_spmd(nc, in_maps, core_ids=list(range(NCORES)))
    results = res.results

    images_encoded = np.empty((B, OUT), np.float32)
    hidden_encoded = np.empty((B, OUT), np.float32)
    for core in range(NCORES):
        sl = slice(core * BS, (core + 1) * BS)
        oi = results[core]["oimg"]
        oh = results[core]["ohid"]
        images_encoded[sl, : OUT // 2] = oi[0:64]
        images_encoded[sl, OUT // 2 :] = oi[64:128]
        hidden_encoded[sl, : OUT // 2] = oh[0:64]
        hidden_encoded[sl, OUT // 2 :] = oh[64:128]
    return images_encoded, hidden_encoded


# revision 19
# speedup vs baseline: 1.0421x; 1.0273x over previous
"""Trainium2 Bass kernel for nn_Listener (LSTM listener + dense encoders).

Reference computation (per full batch B=512):
    emb = embed_table[message]                       # [B, T, 512]
    LSTM over T=128 steps, HIDDEN=1024:
        gated = [x_t, h] @ W_cell + b_cell           # [B, 4096] (i, g, f, o)
        f = sigmoid(f + 1); c = f*c + sigmoid(i)*tanh(g); h = sigmoid(o)*tanh(c)
    images_encoded = images @ W_img + b_img          # [B, 1024]
    hidden_encoded = h @ W_hid + b_hid               # [B, 1024]
    returns (images_encoded, hidden_encoded)

Strategy (8 NeuronCores, data-parallel over batch, 64 rows/core):
  * Embedding lookup + x-projection fold into one table:
        M2 = embed_table @ W_cell[:512] + b_cell  (+1 on the f columns)
    stored bf16; per step the rows are gathered by token id and injected
    into PSUM with a k=128 bf16 selection matmul that also seeds the
    accumulation groups.
  * Per-core batch is 64 = half the PE output partitions; hidden units
    split in half across PSUM partition ranges (partitions 0:64 =
    (batch, units 0:512), 64:128 = (batch, units 512:1024)); the two
    column-groups of the PE run concurrently (verified in trace:
    216 ns/pair at N=512 bf16).
  * Gate bank order is (g, i, f, o) and the f bank runs as two
    half-width streams, so c = f*c + i*g and tanh(c) complete during the
    o bank's matmuls; only sigmoid(o)*tanh(c) + transpose + cast remain
    on the step-boundary critical path (split into column halves, with
    k-chunk order 0,4,1,5,... so the next step starts after half a cast).
  * X-injection matmuls for step t+1 issue right after step t's pairs,
    filling the PE during the epilogue.
  * h stays bf16 end-to-end: bf16 PE transposes into a bf16 PSUM tile,
    one 2x-rate DVE cast back to SBUF.
"""

import os
import numpy as np

B, T = 512, 128
HIDDEN = 1024
VOCAB = 1024
EMB = 512
OUT = 1024
D_IMG = 2048
NCORES = 8
BS = B // NCORES  # 64 batch rows per core
HH = HIDDEN // 2  # 512 = per-half hidden units

_CACHE = {}

# gate order inside the packed tables / PSUM banks: (g, i, f, o)
GATE_PERM = [1, 0, 2, 3]  # indices into natural (i, g, f, o)
BANK_G, BANK_I, BANK_F, BANK_O = 0, 1, 2, 3
CI_ORDER = [0, 4, 1, 5, 2, 6, 3, 7]  # k-chunks: first 4 need tp blocks 0,1


def _build_nc(n_steps: int):
    import concourse.bass as bass
    import concourse.mybir as mybir
    from concourse import bacc, tile

    f32 = mybir.dt.float32
    f32r = mybir.dt.float32r
    bf16 = mybir.dt.bfloat16
    i32 = mybir.dt.int32
    AF = mybir.ActivationFunctionType

    nc = bacc.Bacc("TRN2", target_bir_lowering=False, debug=False)

    m2p_d = nc.declare_dram_parameter("m2p", [2 * VOCAB, HH * 4], bf16, isOutput=False)
    wh_d = nc.declare_dram_parameter("wh", [HIDDEN, 4 * HIDDEN], bf16, isOutput=False)
    msg2_d = nc.declare_dram_parameter("msg2", [2 * BS, T], i32, isOutput=False)
    sfull_d = nc.declare_dram_parameter("sfull", [2 * BS, 2 * BS], bf16, isOutput=False)
    ident_d = nc.declare_dram_parameter("ident", [128, 128], f32, isOutput=False)
    identb_d = nc.declare_dram_parameter("identb", [128, 128], bf16, isOutput=False)
    imgs_d = nc.declare_dram_parameter("imgs", [128, D_IMG // 2], f32, isOutput=False)
    wimg_d = nc.declare_dram_parameter("wimg", [D_IMG, OUT], bf16, isOutput=False)
    whid_d = nc.declare_dram_parameter("whid", [HIDDEN, OUT], bf16, isOutput=False)
    o2_d = nc.declare_dram_parameter("o2", [2, 128], f32r, isOutput=False)
    bimg2_d = nc.declare_dram_parameter("bimg2", [2, OUT // 2], f32r, isOutput=False)
    bhid2_d = nc.declare_dram_parameter("bhid2", [2, OUT // 2], f32r, isOutput=False)
    oimg_d = nc.declare_dram_parameter("oimg", [128, OUT // 2], f32, isOutput=True)
    ohid_d = nc.declare_dram_parameter("ohid", [128, OUT // 2], f32, isOutput=True)

    with tile.TileContext(nc) as tc:
        with (
            tc.tile_pool(name="wpool", bufs=1) as wpool,
            tc.tile_pool(name="const", bufs=1) as cpool,
            tc.tile_pool(name="xg", bufs=3) as xgpool,
            tc.tile_pool(name="state", bufs=2) as stpool,
            tc.tile_pool(name="act", bufs=1) as apool,
            tc.tile_pool(name="outs", bufs=1) as opool,
            tc.tile_pool(name="psum", bufs=1, space="PSUM") as pspool,
        ):
            # ---- constants / small inputs ----
            msg2 = cpool.tile([2 * BS, T], i32, tag="msg2")
            nc.sync.dma_start(msg2[:], msg2_d[:])
            sfull = cpool.tile([2 * BS, 2 * BS], bf16, tag="sfull")
            nc.sync.dma_start(sfull[:], sfull_d[:])
            ident = cpool.tile([128, 128], f32, tag="ident")
            nc.sync.dma_start(ident[:], ident_d[:])
            identb = cpool.tile([128, 128], bf16, tag="identb")
            nc.sync.dma_start(identb[:], identb_d[:])
            o2 = cpool.tile([2, 128], f32r, tag="o2")
            nc.sync.dma_start(o2[:], o2_d[:])
            bimg2 = cpool.tile([2, OUT // 2], f32r, tag="bimg2")
            nc.sync.dma_start(bimg2[:], bimg2_d[:])
            bhid2 = cpool.tile([2, OUT // 2], f32r, tag="bhid2")
            nc.sync.dma_start(bhid2[:], bhid2_d[:])

            # ---- W_h resident in SBUF: 8 chunks of [128, 4096] ----
            wh_sb = []
            for ci in range(8):
                wt = wpool.tile([128, 4 * HIDDEN], bf16, tag=f"wh{ci}")
                nc.sync.dma_start(wt[:], wh_d[128 * ci : 128 * (ci + 1), :])
                wh_sb.append(wt)
            # encoder weights resident too (DMA overlaps the recurrence)
            whid_sb = []
            for ci in range(8):
                wt = wpool.tile([128, OUT], bf16, tag=f"whid{ci}")
                nc.sync.dma_start(wt[:], whid_d[128 * ci : 128 * (ci + 1), :])
                whid_sb.append(wt)
            wimg_sb = []
            for ci in range(16):
                wt = wpool.tile([128, OUT], bf16, tag=f"wimg{ci}")
                nc.sync.dma_start(wt[:], wimg_d[128 * ci : 128 * (ci + 1), :])
                wimg_sb.append(wt)

            # ---- images transposed up front (also PE warmup) ----
            # imgs packed [128, 1024]: partitions 0:64 = batch x feats 0:1024,
            # partitions 64:128 = batch x feats 1024:2048. Cast to bf16 first
            # so the transposes share the bf16 "tp" PSUM tile with the loop.
            imgs = cpool.tile([128, D_IMG // 2], f32, tag="imgs")
            nc.sync.dma_start(imgs[:], imgs_d[:])
            imgsb = cpool.tile([128, D_IMG // 2], bf16, tag="imgsb")
            nc.vector.tensor_copy(imgsb[:], imgs[:])
            imT = cpool.tile([128, D_IMG // 2], bf16, tag="imT")
            for half in range(2):
                tpw = pspool.tile([128, 8 * BS], bf16, tag="tp")
                for q in range(4):
                    qq = 4 * half + q
                    nc.tensor.transpose(
                        out=tpw[:, 128 * q : 128 * (q + 1)],
                        in_=imgsb[:, 128 * qq : 128 * (qq + 1)],
                        identity=identb[:],
                    )
                nc.vector.tensor_copy(imT[:, 512 * half : 512 * (half + 1)], tpw[:])

            # ---- LSTM state init ----
            c_prev = stpool.tile([128, HH], f32, tag="c")
            nc.vector.memset(c_prev[:], 0.0)
            hT_cur = None

            def hT_sl(hT, ci):
                # packed-transpose layout: pair q holds chunk q (cols 0:64)
                # and chunk q+4 (cols 64:128) at col block 128*q
                q, hi = (ci - 4, 64) if ci >= 4 else (ci, 0)
                return hT[:, 128 * q + hi : 128 * q + hi + 64]

            def imT_sl(ci):
                q, hi = (ci - 8, 64) if ci >= 8 else (ci, 0)
                return imT[:, 128 * q + hi : 128 * q + hi + 64]

            def gather(t):
                xg = xgpool.tile([2 * BS, 4 * HH], bf16, tag="xg", name=f"xg_{t}")
                nc.gpsimd.indirect_dma_start(
                    out=xg[:],
                    out_offset=None,
                    in_=m2p_d[:],
                    in_offset=bass.IndirectOffsetOnAxis(ap=msg2[:, t : t + 1], axis=0),
                )
                return xg

            # Gate PSUM tiles: g and i full banks; f and o split into two
            # half-written full-bank tiles each, so a half-stream's consumer
            # only depends on that half's matmuls (deps are tile-granular) and
            # PE writes never share a bank with ACT/DVE reads.
            # STREAMS: (key, gate, gate_c0, gate_c1, out_c0)
            STREAMS = (
                ("g", 0, 0, HH, 0),
                ("i", 1, 0, HH, 0),
                ("f1", 2, 0, HH // 2, 0),
                ("f2", 2, HH // 2, HH, 0),
                ("o1", 3, 0, HH // 2, 0),
                ("o2", 3, HH // 2, HH, 0),
            )

            def alloc_banks(t):
                return {
                    k: pspool.tile([128, HH], f32, tag=f"gp_{k}", name=f"gp_{k}_{t}")
                    for k in ("g", "i", "f1", "f2", "o1", "o2")
                }

            X_FLOOR = {"i": 0.0086, "f1": 0.0088, "f2": 0.009, "o1": 0.0092}

            def x_inject(gpb, xg, t, stop, base=None):
                # seeds every bank (start=True over all 128 partitions);
                # f/o injects are sim-floored past transp01 so they don't
                # queue-block the transposes -> cast1 -> first-pair chain
                for k, gate, c0, c1, oc0 in STREAMS:
                    fl = X_FLOOR.get(k) if base is not None else None
                    with tc.tile_wait_until(0 if fl is None else base + fl,
                                            enable=fl is not None):
                        nc.tensor.matmul(
                            out=gpb[k][:, oc0 : oc0 + (c1 - c0)],
                            lhsT=sfull[:],
                            rhs=xg[:, HH * gate + c0 : HH * gate + c1],
                            start=True,
                            stop=stop,
                            skip_group_check=True,
                        )

            # ---- prologue: gathers + step-0 gates (pure x contribution) ----
            xg_t = [None] * (n_steps + 2)
            xg_t[0] = gather(0)
            xg_t[1] = gather(1)
            gpb_cur = alloc_banks(0)
            x_inject(gpb_cur, xg_t[0], 0, stop=True)

            # ---- recurrence ----
            for t in range(n_steps):
                last_t = t == n_steps - 1
                if t + 2 <= n_steps - 1:
                    xg_t[t + 2] = gather(t + 2)

                gpb = gpb_cur
                if t > 0:
                    # h @ W_h pairs: stream order (g, i, f1, f2, o1, o2) so
                    # the c chain finishes during the o streams. Sim-floored
                    # past the previous tail's cast1 so the scheduler emits
                    # the tail (transposes/casts/X) ahead of the stream.
                    ctx_pairs = tc.tile_wait_until(0.022 * t + 0.009)
                    ctx_pairs.__enter__()
                    for k, gate, c0, c1, oc0 in STREAMS:
                        w0 = 1024 * gate
                        for ci in CI_ORDER:
                            last = ci == CI_ORDER[-1]
                            lhs = hT_sl(hT_cur, ci)
                            nc.tensor.matmul(
                                out=gpb[k][0:64, oc0 : oc0 + (c1 - c0)],
                                lhsT=lhs,
                                rhs=wh_sb[ci][:, w0 + c0 : w0 + c1],
                                start=False,
                                stop=last,
                                skip_group_check=True,
                            )
                            nc.tensor.matmul(
                                out=gpb[k][64:128, oc0 : oc0 + (c1 - c0)],
                                lhsT=lhs,
                                rhs=wh_sb[ci][:, w0 + 512 + c0 : w0 + 512 + c1],
                                start=False,
                                stop=last,
                                skip_group_check=True,
                            )

                if t > 0:
                    ctx_pairs.__exit__(None, None, None)

                # X injection for step t+1 (fills the PE during the epilogue;
                # each bank's inject waits only for its step-t activation)
                if not last_t:
                    gpb_next = alloc_banks(t + 1)
                    x_inject(gpb_next, xg_t[t + 1], t + 1, stop=False,
                             base=0.022 * (t + 1))

                # ---- epilogue: gates -> c, h ----
                # ACT issue order is forced with sim-dispatch floors on a
                # 22 us grid (scheduling-pass only, no runtime cost): the
                # scheduler's cost model misjudges the PE stream (it doesn't
                # model col-group concurrency) and would otherwise hoist the
                # data-chained tanh ops ahead of the stop-gated sigmoids.
                base = 0.022 * (t + 1)
                tg = apool.tile([128, HH], f32, tag="tg")
                with tc.tile_wait_until(base):
                    nc.scalar.activation(tg[:], gpb["g"][:], AF.Tanh)
                si = apool.tile([128, HH], f32, tag="si")
                with tc.tile_wait_until(base + 0.0015):
                    nc.scalar.activation(si[:], gpb["i"][:], AF.Sigmoid)
                m1 = apool.tile([128, HH], f32, tag="m1")
                nc.vector.tensor_mul(m1[:], si[:], tg[:])
                sf = apool.tile([128, HH], f32, tag="sf")
                c_new = stpool.tile([128, HH], f32, tag="c")
                tc_ = apool.tile([128, HH], bf16, tag="tc")
                so = apool.tile([128, HH], bf16, tag="so")
                h = apool.tile([128, HH], bf16, tag="h")
                cm = apool.tile([128, HH], f32, tag="cm")
                HQ = HH // 2
                for hf in range(2):
                    sl = slice(HQ * hf, HQ * (hf + 1))
                    with tc.tile_wait_until(base + 0.003 + 0.0015 * hf):
                        nc.scalar.activation(
                            sf[:, sl], gpb[f"f{hf + 1}"][:, 0:HQ], AF.Sigmoid
                        )
                    nc.vector.tensor_mul(cm[:, sl], sf[:, sl], c_prev[:, sl])
                    nc.vector.tensor_add(c_new[:, sl], cm[:, sl], m1[:, sl])
                # interleave tanh(c) and sigmoid(o) halves so the h halves
                # pipeline: tc1, so1, tc2, so2 on the ACT queue
                for hf in range(2):
                    sl = slice(HQ * hf, HQ * (hf + 1))
                    with tc.tile_wait_until(base + 0.006 + 0.003 * hf):
                        nc.scalar.activation(tc_[:, sl], c_new[:, sl], AF.Tanh)
                    with tc.tile_wait_until(base + 0.0075 + 0.003 * hf):
                        nc.scalar.activation(
                            so[:, sl], gpb[f"o{hf + 1}"][:, 0:HQ], AF.Sigmoid
                        )
                    nc.vector.tensor_mul(h[:, sl], so[:, sl], tc_[:, sl])

                # h -> h^T: 4 bf16 PE transposes + 2 half casts; the next
                # step's first 4 pairs (chunks 0,4,1,5) need only cast half 1.
                tp = pspool.tile([128, 8 * BS], bf16, tag="tp")
                hT_next = stpool.tile([128, 8 * BS], bf16, tag="hT")
                for half in range(2):
                    for q in (2 * half, 2 * half + 1):
                        nc.tensor.transpose(
                            out=tp[:, 128 * q : 128 * (q + 1)],
                            in_=h[:, 128 * q : 128 * (q + 1)],
                            identity=identb[:],
                        )
                    if half == 0:
                        nc.vector.tensor_copy(hT_next[:, 0:128], tp[:, 0:128])
                        nc.vector.tensor_copy(hT_next[:, 128:256], tp[:, 128:256])
                    else:
                        nc.vector.tensor_copy(hT_next[:, 256:512], tp[:, 256:512])

                c_prev = c_new
                hT_cur = hT_next
                if not last_t:
                    gpb_cur = gpb_next

            # ---- hidden encoder: out = h @ W_hid + b_hid ----
            ohp = pspool.tile([128, OUT // 2], f32, tag="gp_g")
            nc.tensor.matmul(
                out=ohp[:], lhsT=o2[:], rhs=bhid2[:],
                start=True, stop=False, skip_group_check=True,
            )
            for ci in range(8):
                last = ci == 7
                lhs = hT_sl(hT_cur, ci)
                nc.tensor.matmul(
                    out=ohp[0:64, :], lhsT=lhs, rhs=whid_sb[ci][:, 0:512],
                    start=False, stop=last, skip_group_check=True,
                )
                nc.tensor.matmul(
                    out=ohp[64:128, :], lhsT=lhs, rhs=whid_sb[ci][:, 512:1024],
                    start=False, stop=last, skip_group_check=True,
                )
            ohid_sb = opool.tile([128, OUT // 2], f32, tag="ohid")
            nc.vector.tensor_copy(ohid_sb[:], ohp[:])
            nc.sync.dma_start(ohid_d[:], ohid_sb[:])

            # ---- images encoder: out = images @ W_img + b_img ----
            oip = pspool.tile([128, OUT // 2], f32, tag="gp_i")
            nc.tensor.matmul(
                out=oip[:], lhsT=o2[:], rhs=bimg2[:],
                start=True, stop=False, skip_group_check=True,
            )
            for ci in range(16):
                last = ci == 15
                lhs = imT_sl(ci)
                nc.tensor.matmul(
                    out=oip[0:64, :], lhsT=lhs, rhs=wimg_sb[ci][:, 0:512],
                    start=False, stop=last, skip_group_check=True,
                )
                nc.tensor.matmul(
                    out=oip[64:128, :], lhsT=lhs, rhs=wimg_sb[ci][:, 512:1024],
                    start=False, stop=last, skip_group_check=True,
                )
            oimg_sb = opool.tile([128, OUT // 2], f32, tag="oimg")
            nc.vector.tensor_copy(oimg_sb[:], oip[:])
            nc.sync.dma_start(oimg_d[:], oimg_sb[:])

    nc.compile()
    return nc


def _host_prep(images, embed_table, W_cell, b_cell, W_img, b_img, W_hid, b_hid,
               message):
    """Builds the per-core input maps (all host-side preprocessing)."""
    from ml_dtypes import bfloat16

    W_x = W_cell[:EMB]          # [512, 4096]
    W_h = W_cell[EMB:]          # [1024, 4096]

    M2 = embed_table.astype(np.float32) @ W_x + b_cell  # [1024, 4096]
    M2[:, 2 * HIDDEN : 3 * HIDDEN] += 1.0  # fold the f-gate +1.0
    # permute gate blocks to bank order (g, i, f, o)
    M2 = np.concatenate(
        [M2[:, 1024 * p : 1024 * (p + 1)] for p in GATE_PERM], axis=1
    )
    W_hp = np.concatenate(
        [W_h[:, 1024 * p : 1024 * (p + 1)] for p in GATE_PERM], axis=1
    ).astype(bfloat16)
    # row 2v+h = [g_h, i_h, f_h, o_h] halves of vocab row v
    M2p = np.ascontiguousarray(
        M2.reshape(VOCAB, 4, 2, HH).transpose(0, 2, 1, 3).reshape(2 * VOCAB, 4 * HH)
    ).astype(bfloat16)

    sfull = np.zeros((2 * BS, 2 * BS), np.float32)
    for m in range(BS):
        sfull[2 * m, m] = 1.0
        sfull[2 * m + 1, BS + m] = 1.0
    sfull = sfull.astype(bfloat16)

    ident = np.eye(128, dtype=np.float32)
    identb = ident.astype(bfloat16)

    o2 = np.zeros((2, 128), np.float32)
    o2[0, 0:64] = 1.0
    o2[1, 64:128] = 1.0

    W_img_b = W_img.astype(bfloat16)
    W_hid_b = W_hid.astype(bfloat16)
    bimg2 = np.stack([b_img[: OUT // 2], b_img[OUT // 2 :]]).astype(np.float32)
    bhid2 = np.stack([b_hid[: OUT // 2], b_hid[OUT // 2 :]]).astype(np.float32)

    in_maps = []
    for core in range(NCORES):
        sl = slice(core * BS, (core + 1) * BS)
        msg = message[sl]  # [64, T] int32
        msg2 = np.empty((2 * BS, T), np.int32)
        msg2[0::2] = 2 * msg
        msg2[1::2] = 2 * msg + 1
        in_maps.append(
            {
                "m2p": M2p,
                "wh": W_hp,
                "msg2": msg2,
                "sfull": sfull,
                "ident": ident,
                "identb": identb,
                "imgs": np.concatenate(
                    [images[sl, : D_IMG // 2], images[sl, D_IMG // 2 :]], axis=0
                ),
                "wimg": W_img_b,
                "whid": W_hid_b,
                "o2": o2,
                "bimg2": bimg2,
                "bhid2": bhid2,
            }
        )
    return in_maps


def kernel(images, embed_table, W_cell, b_cell, W_img, b_img, W_hid, b_hid,
           message):
    import sys
    if "/opt/trn_rl_repo" not in sys.path:
        sys.path.insert(0, "/opt/trn_rl_repo")
    from concourse.bass_utils import run_bass_kernel_spmd

    images = np.asarray(images, np.float32)
    embed_table = np.asarray(embed_table, np.float32)
    W_cell = np.asarray(W_cell, np.float32)
    b_cell = np.asarray(b_cell, np.float32)
    W_img = np.asarray(W_img, np.float32)
    b_img = np.asarray(b_img, np.float32)
    W_hid = np.asarray(W_hid, np.float32)
    b_hid = np.asarray(b_hid, np.float32)
    message = np.asarray(message, np.int32)

    n_steps = T
    if "nc" not in _CACHE or _CACHE.get("n_steps") != n_steps:
        _CACHE["nc"] = _build_nc(n_steps)
        _CACHE["n_steps"] = n_steps
    nc = _CACHE["nc"]

    in_maps = _host_prep(
        images, embed_table, W_cell, b_cell, W_img, b_img, W_hid, b_hid, message
    )
    res = run_bass_kernel_spmd(nc, in_maps, core_ids=list(range(NCORES)))
    results = res.results

    images_encoded = np.empty((B, OUT), np.float32)
    hidden_encoded = np.empty((B, OUT), np.float32)
    for core in range(NCORES):
        sl = slice(core * BS, (core + 1) * BS)
        oi = results[core]["oimg"]
        oh = results[core]["ohid"]
        images_encoded[sl, : OUT // 2] = oi[0:64]
        images_encoded[sl, OUT // 2 :] = oi[64:128]
        hidden_encoded[sl, : OUT // 2] = oh[0:64]
        hidden_encoded[sl, OUT // 2 :] = oh[64:128]
    return images_encoded, hidden_encoded
